# revision 26
# baseline (speedup 1.0000x reference)
"""GCNII forward on 8 TRN2 NeuronCores (self-contained).

Strategy (1D row partitioning per sharding hint):
- nodes sharded 2500/core (padded 2560); edges assigned to the core owning dst.
- per layer: ht = dinv*h exchanged as fp8(e4m3) via AllGathers into a
  SHARED DRAM table [20480,1024]; the table is split into NBLK row-blocks so
  the collective for block b can start as soon as the tiles producing it are
  done (overlaps with the remaining tiles' compute).
- each core gathers all of a dst-tile's source rows with ONE dma_gather
  (Q7/mlp ucode, 16 SDMA engines), then scatter-adds them into per-dst-tile
  PSUM via one-hot fp8 matmuls in DoubleRow perf mode (2 edge-chunks per
  instruction), computes z = 0.9*dinv*agg + 0.1*h0 (h0 kept in SBUF) and the
  layer GEMM z @ ((1-b)I + b*W) in bf16 with the identity residual folded
  into the weights on the host.
- self-loops are real edges; gcn_norm folded into per-node dinv scaling.
"""
import math
import numpy as np
import ml_dtypes

import concourse.bass as bass
import concourse.mybir as mybir
import concourse.tile as tile
from concourse import bacc, library_config
from concourse.bass_utils import run_bass_kernel_spmd
from concourse.masks import make_identity

# problem constants (hardcoded per contract)
N, E = 20000, 320000
F_IN, H, C, L = 512, 1024, 64, 8
ALPHA, THETA = 0.1, 0.5
NCORES = 8
SH = N // NCORES          # 2500 real rows per core
SHP = 2560                # padded rows per core (20*128)
P = 128
NT = SHP // P             # 20 dst tiles per core
KF = F_IN // P            # 4 k-tiles for W1
KH = H // P               # 8 k-tiles for H
NBLK = 2                  # table row-blocks (collective chunks per layer)
GMAX = 8                  # max chunks per dma_gather (1024 rows: HW ring limit)
RB = SHP // NBLK          # rows per block per core
TPB = NT // NBLK          # tiles per block
VB = NCORES * RB          # table rows per block
V2 = NCORES * SHP         # total table rows
# emit the block-b collective after this tile's gather (overlap knob);
# block NBLK-1's collective is emitted right after the last tile.
EMIT_AFTER = {b: min(b * TPB + TPB + 2, NT - 1) for b in range(NBLK - 1)}

f32 = mybir.dt.float32
f32r = mybir.dt.float32r
bf16 = mybir.dt.bfloat16
f8 = mybir.dt.float8e4
i32 = mybir.dt.int32
i16 = mybir.dt.int16

np_f8 = ml_dtypes.float8_e4m3fn
np_bf16 = ml_dtypes.bfloat16

_cache = {}


def _preprocess(x, edge_index, W1, b1, Wg, W2, b2):
    src = np.asarray(edge_index[0], dtype=np.int64)
    dst = np.asarray(edge_index[1], dtype=np.int64)
    # self loops
    loops = np.arange(N, dtype=np.int64)
    src = np.concatenate([src, loops])
    dst = np.concatenate([dst, loops])
    deg = np.bincount(dst, minlength=N).astype(np.float32)
    dinv = 1.0 / np.sqrt(np.maximum(deg, 1.0))

    core = dst // SH
    d_loc = dst - core * SH
    tl = d_loc // P
    slot = d_loc % P
    gid = core * NT + tl
    order = np.argsort(gid, kind="stable")
    gid_s = gid[order]
    src_s = src[order]
    slot_s = slot[order]
    # rank within group
    counts = np.bincount(gid_s, minlength=NCORES * NT)
    starts = np.concatenate([[0], np.cumsum(counts)[:-1]])
    j = np.arange(len(gid_s)) - starts[gid_s]
    nch = int(math.ceil(counts.max() / P))
    nch += nch % 2            # even for DoubleRow pairing
    NIDX = nch * P
    IC = NIDX // 16
    c_idx = j // P
    p_idx = j % P
    # table row for a source node, with the NBLK row-block layout
    sc = src_s // SH
    r = src_s % SH
    blk = r // RB
    s_tab = (blk * VB + sc * RB + (r - blk * RB)).astype(np.int64)

    core_s = gid_s // NT
    tl_s = gid_s % NT
    # gather indices: idx16[core, tile, j%16, j//16] = s_tab ; replicated x8
    idx16 = np.zeros((NCORES, NT, 16, IC), dtype=np.int16)
    idx16[core_s, tl_s, j % 16, j // 16] = s_tab.astype(np.int16)
    idx = np.broadcast_to(idx16[:, :, None, :, :],
                          (NCORES, NT, 8, 16, IC)).reshape(NCORES, NT, P, IC)
    idx = np.ascontiguousarray(idx)
    # scatter matrices [core, tile, e_lane, chunk/2, 2, slot] with the
    # 0.9*dinv[dst] z-scaling folded into the coefficients
    dst_s = dst[order]
    S = np.zeros((NCORES, NT, P, nch // 2, 2, P), dtype=np_f8)
    S[core_s, tl_s, p_idx, c_idx // 2, c_idx % 2, slot_s] = \
        (0.9 * dinv[dst_s]).astype(np_f8)

    # per-core dinv columns [P, NT]
    dinv_pad = np.zeros(NCORES * SHP, dtype=np.float32)
    nidx = np.arange(N)
    dinv_pad[(nidx // SH) * SHP + (nidx % SH)] = dinv
    dinvc = dinv_pad.reshape(NCORES, NT, P).transpose(0, 2, 1).copy()  # [c,P,NT]

    # xT shards [F_IN, SHP] padded
    x = np.asarray(x, dtype=np.float32)
    xT = np.zeros((NCORES, F_IN, SHP), dtype=np.float32)
    for c in range(NCORES):
        xT[c, :, :SH] = x[c * SH:(c + 1) * SH].T

    betas = np.log(THETA / np.arange(1.0, L + 1.0, dtype=np.float64) + 1.0)
    Wg = np.asarray(Wg, dtype=np.float64)
    eye = np.eye(H, dtype=np.float64)
    Wt = np.stack([(1.0 - betas[l]) * eye + betas[l] * Wg[l] for l in range(L)])
    # DoubleRow pair packing: Wp[l, k2, p, i, :] = Wt[l, (2*k2+i)*P + p, :]
    Wp = Wt.reshape(L, KH // 2, 2, P, H).transpose(0, 1, 3, 2, 4)
    Wp = np.ascontiguousarray(Wp).astype(np_bf16)

    b1b = np.broadcast_to(np.asarray(b1, np.float32), (P, H)).copy()
    b2b = np.broadcast_to(np.asarray(b2, np.float32), (P, C)).copy()

    in_maps = []
    for c in range(NCORES):
        in_maps.append({
            "xT": xT[c],
            "W1": np.asarray(W1, np.float32),
            "Wt": Wp,
            "W2": np.asarray(W2, np.float32),
            "b1b": b1b,
            "b2b": b2b,
            "dinvc": dinvc[c],
            "idx": idx[c],
            "Smat": S[c],
        })
    return in_maps, nch


def _build(nch):
    NIDX = nch * P
    IC = NIDX // 16
    NC2 = nch // 2
    nc = bacc.Bacc("TRN2", target_bir_lowering=False, debug=False,
                   num_devices=NCORES)
    t_xT = nc.dram_tensor("xT", [F_IN, SHP], f32r, kind="ExternalInput")
    t_W1 = nc.dram_tensor("W1", [F_IN, H], f32r, kind="ExternalInput")
    t_Wt = nc.dram_tensor("Wt", [L, KH // 2, P, 2, H], bf16,
                          kind="ExternalInput")
    t_W2 = nc.dram_tensor("W2", [H, C], f32r, kind="ExternalInput")
    t_b1 = nc.dram_tensor("b1b", [P, H], f32, kind="ExternalInput")
    t_b2 = nc.dram_tensor("b2b", [P, C], f32, kind="ExternalInput")
    t_dinv = nc.dram_tensor("dinvc", [P, NT], f32, kind="ExternalInput")
    t_idx = nc.dram_tensor("idx", [NT, P, IC], i16, kind="ExternalInput")
    t_S = nc.dram_tensor("Smat", [NT, P, NC2, 2, P], f8, kind="ExternalInput")
    t_out = nc.dram_tensor("out", [SHP, C], f32, kind="ExternalOutput")

    exch = [nc.dram_tensor(f"exch{b}", [RB, H], f8) for b in range(NBLK)]
    tables = [nc.dram_tensor(f"tbl{i}", [V2, H], f8, addr_space="Shared")
              for i in range(2)]

    with tile.TileContext(nc) as tc:
        with (
            tc.tile_pool(name="const", bufs=1) as cp,
            tc.tile_pool(name="wpool", bufs=2) as wp,
            tc.tile_pool(name="spool", bufs=2) as sp,
            tc.tile_pool(name="gpool", bufs=3) as gp,
            tc.tile_pool(name="zpool", bufs=2) as zp,
            tc.tile_pool(name="ps_agg", bufs=2, space="PSUM") as pa,
            tc.tile_pool(name="ps_gemm", bufs=1, space="PSUM") as pg,
            tc.tile_pool(name="ps_tr", bufs=2, space="PSUM") as pt,
        ):
            nc.gpsimd.load_library(library_config.mlp)
            ident = cp.tile([P, P], f32, tag="ident")
            make_identity(nc, ident[:])
            dinv_sb = cp.tile([P, NT], f32, tag="dinv")
            nc.sync.dma_start(out=dinv_sb[:], in_=t_dinv[:])
            b1_sb = cp.tile([P, H], f32, tag="b1")
            nc.sync.dma_start(out=b1_sb[:], in_=t_b1[:])
            b2_sb = cp.tile([P, C], f32, tag="b2")
            nc.sync.dma_start(out=b2_sb[:], in_=t_b2[:])
            idx_sb = cp.tile([P, NT * IC], i16, tag="idx")
            for t in range(NT):
                nc.sync.dma_start(out=idx_sb[:, t * IC:(t + 1) * IC],
                                  in_=t_idx[t])
            W2_sb = cp.tile([P, KH * C], f32r, tag="W2")
            for k in range(KH):
                nc.sync.dma_start(out=W2_sb[:, k * C:(k + 1) * C],
                                  in_=t_W2[k * P:(k + 1) * P, :])
            h0s = cp.tile([P, NT * H], bf16, tag="h0s")

            def exchange_tile(ps, t, lnext):
                """relu+dinv scale -> fp8 -> exch block; maybe emit collective."""
                ex_t = zp.tile([P, H], f8, tag="ex")
                nc.scalar.activation(out=ex_t[:], in_=ps[:],
                                     func=mybir.ActivationFunctionType.Relu,
                                     scale=dinv_sb[:, t:t + 1])
                b = t // TPB
                nc.sync.dma_start(
                    out=exch[b][(t - b * TPB) * P:(t - b * TPB + 1) * P, :],
                    in_=ex_t[:])

            def emit_collective(b, lnext):
                nc.gpsimd.collective_compute(
                    "AllGather", mybir.AluOpType.bypass,
                    replica_groups=[list(range(NCORES))],
                    ins=[exch[b].ap().opt()],
                    outs=[tables[lnext % 2][b * VB:(b + 1) * VB].opt()])

            # ---- phase 0: h0 = relu(x@W1 + b1); h0s = 0.1*h0 (SBUF);
            #      table0 = f8(dinv*h0)
            with tc.tile_pool(name="xpool", bufs=1) as xp:
                xT_sb = xp.tile([P, KF * SHP], f32r, tag="xT")
                for k in range(KF):
                    nc.sync.dma_start(out=xT_sb[:, k * SHP:(k + 1) * SHP],
                                      in_=t_xT[k * P:(k + 1) * P, :])
                W1_sb = wp.tile([P, KF * H], f32r, tag="W")
                for k in range(KF):
                    nc.sync.dma_start(out=W1_sb[:, k * H:(k + 1) * H],
                                      in_=t_W1[k * P:(k + 1) * P, :])
                for t in range(NT):
                    ps = pg.tile([P, H], f32, space="PSUM", tag="gemm")
                    for k in range(KF):
                        for nh in range(2):
                            nc.tensor.matmul(
                                out=ps[:, nh * 512:(nh + 1) * 512],
                                lhsT=xT_sb[:, k * SHP + t * P:
                                           k * SHP + (t + 1) * P],
                                rhs=W1_sb[:, k * H + nh * 512:
                                          k * H + (nh + 1) * 512],
                                start=(k == 0), stop=(k == KF - 1))
                    nc.vector.tensor_add(out=ps[:], in0=ps[:], in1=b1_sb[:])
                    nc.scalar.activation(out=h0s[:, t * H:(t + 1) * H],
                                         in_=ps[:],
                                         func=mybir.ActivationFunctionType.Relu,
                                         scale=0.1)
                    exchange_tile(ps, t, 0)
                    for b, at in EMIT_AFTER.items():
                        if at == t:
                            emit_collective(b, 0)
                    if t == NT - 1:
                        for b in range(NBLK):
                            if b not in EMIT_AFTER:
                                emit_collective(b, 0)

            # ---- layers
            for l in range(L):
                tbl = tables[l % 2]
                W_sb = wp.tile([P, KH // 2, 2, H], bf16, tag="W")
                for k2 in range(KH // 2):
                    nc.sync.dma_start(out=W_sb[:, k2], in_=t_Wt[l, k2])
                # software-pipelined emission: tile t's gather+scatter is
                # emitted BEFORE tile t-1's z/transpose/GEMM so the PE never
                # stalls waiting on the DVE z-add (correctness is from data
                # deps, emission order only shapes the per-engine queues).
                aggs = {}

                def front(t):
                    g_all = gp.tile([P, nch, H], f8, tag="g")
                    for c0 in range(0, nch, GMAX):
                        gc = min(GMAX, nch - c0)
                        nc.gpsimd.dma_gather(
                            g_all[:, c0:c0 + gc, :], tbl.ap(),
                            idx_sb[:, t * IC + c0 * 8:
                                   t * IC + (c0 + gc) * 8],
                            gc * P, gc * P, H)
                    if l < L - 1:
                        for b, at in EMIT_AFTER.items():
                            if at == t:
                                emit_collective(b, l + 1)
                    S_sb = sp.tile([P, NC2, 2, P], f8, tag="S")
                    nc.sync.dma_start(out=S_sb[:], in_=t_S[t])
                    agg = pa.tile([P, H], f32, space="PSUM", tag="agg")
                    for c2 in range(NC2):
                        for nh in range(2):
                            nc.tensor.matmul(
                                out=agg[:, nh * 512:(nh + 1) * 512],
                                lhsT=S_sb[:, c2],
                                rhs=g_all[:, 2 * c2:2 * c2 + 2,
                                          nh * 512:(nh + 1) * 512],
                                start=(c2 == 0), stop=(c2 == NC2 - 1),
                                perf_mode=mybir.MatmulPerfMode.DoubleRow)
                    return agg

                for t in range(NT + 1):
                    if t < NT:
                        aggs[t] = front(t)
                    if t == 0:
                        continue
                    t, agg = t - 1, aggs.pop(t - 1)
                    # z = agg + 0.1*h0   (0.9*dinv[dst] folded into S)
                    z = zp.tile([P, H], f32, tag="z")
                    nc.vector.tensor_add(out=z[:], in0=agg[:],
                                         in1=h0s[:, t * H:(t + 1) * H])
                    # transpose z -> zT (bf16, 8 k-tiles)
                    zT = zp.tile([P, KH, P], bf16, tag="zT")
                    for k in range(KH):
                        trp = pt.tile([P, P], f32, space="PSUM", tag="tr")
                        nc.tensor.transpose(out=trp[:],
                                            in_=z[:, k * P:(k + 1) * P],
                                            identity=ident[:])
                        nc.vector.tensor_copy(out=zT[:, k, :], in_=trp[:])
                    ps = pg.tile([P, H], f32, space="PSUM", tag="gemm")
                    for k in range(KH):
                        for nh in range(2):
                            nc.tensor.matmul(
                                out=ps[:, nh * 512:(nh + 1) * 512],
                                lhsT=zT[:, k, :],
                                rhs=W_sb[:, k // 2, k % 2,
                                         nh * 512:(nh + 1) * 512],
                                start=(k == 0), stop=(k == KH - 1))
                    if l < L - 1:
                        exchange_tile(ps, t, l + 1)
                        if t == NT - 1:
                            for b in range(NBLK):
                                if b not in EMIT_AFTER:
                                    emit_collective(b, l + 1)
                    else:
                        # h8 tile -> logits -> log_softmax -> out
                        h8 = zp.tile([P, H], f32, tag="z")
                        nc.scalar.activation(
                            out=h8[:], in_=ps[:],
                            func=mybir.ActivationFunctionType.Relu)
                        hT = zp.tile([P, KH * P], f32r, tag="hT")
                        for k in range(KH):
                            trp = pt.tile([P, P], f32, space="PSUM", tag="tr")
                            nc.tensor.transpose(out=trp[:],
                                                in_=h8[:, k * P:(k + 1) * P],
                                                identity=ident[:])
                            nc.vector.tensor_copy(out=hT[:, k * P:(k + 1) * P],
                                                  in_=trp[:])
                        psl = pt.tile([P, C], f32, space="PSUM", tag="tr")
                        for k in range(KH):
                            nc.tensor.matmul(
                                out=psl[:],
                                lhsT=hT[:, k * P:(k + 1) * P],
                                rhs=W2_sb[:, k * C:(k + 1) * C],
                                start=(k == 0), stop=(k == KH - 1))
                        nc.vector.tensor_add(out=psl[:], in0=psl[:],
                                             in1=b2_sb[:])
                        mx = zp.tile([P, 1], f32, tag="mx")
                        nc.vector.tensor_reduce(out=mx[:], in_=psl[:],
                                                axis=mybir.AxisListType.X,
                                                op=mybir.AluOpType.max)
                        nmx = zp.tile([P, 1], f32, tag="nmx")
                        nc.vector.tensor_scalar(
                            out=nmx[:], in0=mx[:], scalar1=-1.0, scalar2=None,
                            op0=mybir.AluOpType.mult)
                        esb = zp.tile([P, C], f32, tag="esb")
                        se = zp.tile([P, 1], f32, tag="se")
                        nc.scalar.activation(
                            out=esb[:], in_=psl[:],
                            func=mybir.ActivationFunctionType.Exp,
                            bias=nmx[:], accum_out=se[:])
                        lse = zp.tile([P, 1], f32, tag="lse")
                        nc.scalar.activation(
                            out=lse[:], in_=se[:],
                            func=mybir.ActivationFunctionType.Ln)
                        o_t = zp.tile([P, C], f32, tag="ot")
                        nc.vector.tensor_scalar(
                            out=o_t[:], in0=psl[:], scalar1=mx[:],
                            scalar2=lse[:],
                            op0=mybir.AluOpType.subtract,
                            op1=mybir.AluOpType.subtract)
                        nc.sync.dma_start(out=t_out[t * P:(t + 1) * P, :],
                                          in_=o_t[:])
    nc.compile()
    return nc


def kernel(**inputs):
    in_maps, nch = _preprocess(
        inputs["x"], inputs["edge_index"], inputs["W1"], inputs["b1"],
        inputs["Wg"], inputs["W2"], inputs["b2"])
    key = ("nc", nch)
    if key not in _cache:
        _cache[key] = _build(nch)
    nc = _cache[key]
    res = run_bass_kernel_spmd(nc, in_maps, list(range(NCORES)))
    out = np.concatenate(
        [res.results[c]["out"][:SH] for c in range(NCORES)], axis=0)
    return out.astype(np.float32)
